# revision 16
# baseline (speedup 1.0000x reference)
"""Fused single-head cross-attention on 8 TRN2 NeuronCores (Bass/Tile).

Problem: out = (softmax(norm * (xWq+bq)(yWk+bk)^T + adj) @ (yWv+bv)) Wo + bo
Shapes: x,y [4, 2048, 1024], adj [4, 2048, 2048], all weights [1024, 1024].

Sharding: data-parallel over (batch, seq-half) -> 8 shards. Core c handles
batch b=c//2, query rows h*1024..(h+1)*1024 (h=c%2). K/V projections are
split across the core pair (each computes its own t-half) and exchanged
with two pair-wise 2MB AllGathers (K first, then V), hidden under the V/Q
projections. Collectives >2MB hit an RDH buffer cliff (4MB = 107us vs
2MB = 22us), so K and V are exchanged separately.

All matmul operands are bf16 (1 cyc/row on the PE, same as fp32r, but half
the DMA/SBUF/collective bytes); PSUM accumulation is fp32 throughout.
Inputs/weights are cast to bf16 on the host so every DMA moves half the
bytes. Biases stay fp32 (applied via ACT bias port / DVE add).

Attention phase is fully SBUF-resident (K, V, adj, Wo preloaded; zero DMA
during attention), s-block outer:
  pass1: for all 16 t-tiles: att = K^T Q (PSUM), +adj (DVE), exp (ACT,
         bf16 out), denominator accumulated on DVE.
  pass2: numerator accumulated IN PSUM across all 16 t-tiles (4 banks per
         d-half), evacuated once per s-block.
  O-proj per s-block; softmax 1/denom is folded in after the matmul
  (out = (Wo^T num) * recip + bo), so pass2 needs no pre-scaling.
softmax max-subtraction is skipped: logits are O(1) by construction.
"""
import sys

if "/opt/trn_rl_repo" not in sys.path:
    sys.path.insert(0, "/opt/trn_rl_repo")

import numpy as np
import ml_dtypes

import concourse.bass as bass
import concourse.bass_isa as bass_isa
import concourse.tile as tile
from concourse import bacc, mybir
from concourse.bass_utils import run_bass_kernel_spmd

P = 128
D = 1024
S = 2048
SC = 1024            # per-core query rows; also per-core K/V t-half
DC = D // P          # 8 feature chunks
SB = 512             # matmul moving free dim
NSB = SC // SB       # 2 s blocks
NG = S // P          # 16 global t-tiles
NTT = SC // P        # 8 local t-tiles (own half)
NORM = 1.0 / 32.0
GROUPS = [[0, 1], [2, 3], [4, 5], [6, 7]]

F32 = mybir.dt.float32
BF16 = mybir.dt.bfloat16
ID = mybir.ActivationFunctionType.Identity
EXP = mybir.ActivationFunctionType.Exp

_CACHE = {}


def _mm(nc, ps, lhsT, rhs, start, stop):
    nc.tensor.matmul(ps, lhsT=lhsT, rhs=rhs, start=start, stop=stop)


def build_nc():
    nc = bacc.Bacc("TRN2", target_bir_lowering=False, debug=False, num_devices=8)

    xT = nc.dram_tensor("xT", [D, SC], BF16, kind="ExternalInput")
    yT = nc.dram_tensor("yT", [D, SC], BF16, kind="ExternalInput")  # own t-half
    adjT = nc.dram_tensor("adjT", [S, SC], BF16, kind="ExternalInput")
    # weights pre-tiled on host: Wx_t[dt][p][c][col] = Wx[c*P+p, dt*P+col]
    Wq = nc.dram_tensor("Wq", [DC, P, DC, P], BF16, kind="ExternalInput")
    Wk = nc.dram_tensor("Wk", [DC, P, DC, P], BF16, kind="ExternalInput")
    Wo = nc.dram_tensor("Wo", [DC, P, DC, P], BF16, kind="ExternalInput")
    # Wv pre-tiled as rhs: Wv_t[db][p][c][col] = Wv[c*P+p, db*SB+col]
    Wv = nc.dram_tensor("Wv", [2, P, DC, SB], BF16, kind="ExternalInput")
    bq = nc.dram_tensor("bq", [P, DC], F32, kind="ExternalInput")
    bk = nc.dram_tensor("bk", [P, DC], F32, kind="ExternalInput")
    bv = nc.dram_tensor("bv", [1, D], F32, kind="ExternalInput")
    bo = nc.dram_tensor("bo", [P, DC], F32, kind="ExternalInput")
    outT = nc.dram_tensor("outT", [D, SC], F32, kind="ExternalOutput")

    # local K/V t-halves + pair-gathered full-S tensors (bf16, 2MB each)
    kT_loc = nc.dram_tensor("kT_loc", [D, SC], BF16)
    v_loc = nc.dram_tensor("v_loc", [SC, D], BF16)
    kT_all = nc.dram_tensor("kT_all", [2, D, SC], BF16)
    v_all = nc.dram_tensor("v_all", [2, SC, D], BF16)

    xT_r = xT.rearrange("(c p) s -> p c s", p=P)
    yT_r = yT.rearrange("(c p) t -> p c t", p=P)
    adj_r = adjT.rearrange("(g p) s -> p g s", p=P)
    kT_all_r = kT_all.rearrange("r (c p) t -> r p c t", p=P)
    v_all_r = v_all.rearrange("r (j p) d -> r p j d", p=P)
    Wk_r = Wk.rearrange("dt p c col -> p dt c col")
    Wq_r = Wq.rearrange("dt p c col -> p dt c col")
    Wo_r = Wo.rearrange("dt p c col -> p dt c col")
    Wv_r = Wv.rearrange("db p c col -> p db c col")

    with tile.TileContext(nc) as tc:
        with (
            nc.allow_low_precision(reason="bf16 attention, fp32 accumulation"),
            tc.tile_pool(name="res", bufs=1) as res,
        ):
            # ---- resident tiles --------------------------------------
            QT_sb = res.tile([P, DC, SC], BF16, name="QT_sb")
            adj_sb = res.tile([P, NG, SC], BF16, name="adj_sb")
            wo_res = res.tile([P, DC, DC, P], BF16, name="wo_res")
            denacc = res.tile([P, NSB, SB], F32, name="denacc")
            dsum = res.tile([P, SB], F32, name="dsum")
            recip_sb = res.tile([1, NSB, SB], F32, name="recip_sb")
            rb = res.tile([P, NSB, SB], F32, name="rb")
            bv_bc = res.tile([P, D], F32, name="bv_bc")
            bq_sb = res.tile([P, DC], F32, name="bq_sb")
            bk_sb = res.tile([P, DC], F32, name="bk_sb")
            bo_sb = res.tile([P, DC], F32, name="bo_sb")
            bv_sb = res.tile([1, D], F32, name="bv_sb")
            nc.sync.dma_start(out=bk_sb[:], in_=bk[:])
            nc.sync.dma_start(out=bv_sb[:], in_=bv[:])
            nc.sync.dma_start(out=bq_sb[:], in_=bq[:])
            nc.sync.dma_start(out=bo_sb[:], in_=bo[:])
            nc.gpsimd.partition_broadcast(bv_bc[:], bv_sb[0:1, :], channels=P)

            with (
                tc.tile_pool(name="qkv_in", bufs=1) as qkvp,
                tc.tile_pool(name="kv_out", bufs=3) as kvo,
                tc.tile_pool(name="qkv_ps", bufs=3, space="PSUM") as qps,
            ):
                yT_sb = qkvp.tile([P, DC, SC], BF16, name="yT_sb")
                xT_sb = qkvp.tile([P, DC, SC], BF16, name="xT_sb")
                wk_res = qkvp.tile([P, DC, DC, P], BF16, name="wk_res")
                wq_res = qkvp.tile([P, DC, DC, P], BF16, name="wq_res")
                wv_res = qkvp.tile([P, 2, DC, SB], BF16, name="wv_res")
                # DMA dispatch is FIFO *per ring* (~2us fixed + bytes/340GB/s
                # each, serial). Three rings exist: sync (SP HWDGE), scalar
                # (ACT HWDGE), gpsimd (SWDGE). Spread loads so no ring's
                # backlog ever gates compute:
                #   sync:   yT, wk, wv (K/V-proj critical path), K/V gathers,
                #           out stores
                #   scalar: xT, wq, adj, wo (needed later; dispatched up
                #           front, drains in the background)
                #   gpsimd: kt/vt stores feeding the collectives
                nc.sync.dma_start(out=yT_sb[:, :, 0:SB], in_=yT_r[:, :, 0:SB])
                nc.sync.dma_start(out=wk_res[:, 0], in_=Wk_r[:, 0])
                nc.sync.dma_start(out=yT_sb[:, :, SB:SC], in_=yT_r[:, :, SB:SC])
                nc.sync.dma_start(out=wk_res[:, 1:4], in_=Wk_r[:, 1:4])
                nc.sync.dma_start(out=wk_res[:, 4:8], in_=Wk_r[:, 4:8])
                nc.sync.dma_start(out=wv_res[:], in_=Wv_r[:])
                for i in range(2):
                    nc.sync.dma_start(
                        out=xT_sb[:, 4 * i : 4 * (i + 1), :],
                        in_=xT_r[:, 4 * i : 4 * (i + 1), :],
                    )
                for i in range(2):
                    nc.sync.dma_start(
                        out=wq_res[:, 4 * i : 4 * (i + 1)],
                        in_=Wq_r[:, 4 * i : 4 * (i + 1)],
                    )
                for i in range(2):
                    nc.sync.dma_start(
                        out=adj_sb[:, 8 * i : 8 * (i + 1), :],
                        in_=adj_r[:, 8 * i : 8 * (i + 1), :],
                    )
                for i in range(2):
                    nc.sync.dma_start(
                        out=wo_res[:, 4 * i : 4 * (i + 1)],
                        in_=Wo_r[:, 4 * i : 4 * (i + 1)],
                    )

                # ---- phase K: K^T = Wk^T y^T + bk (own t-half) -------
                for dt in range(DC):
                    for tb in range(NSB):
                        ps = qps.tile([P, SB], F32, name="k_ps", tag="qkvps")
                        for c in range(DC):
                            _mm(
                                nc, ps[:],
                                wk_res[:, dt, c, :],
                                yT_sb[:, c, tb * SB : (tb + 1) * SB],
                                c == 0, c == DC - 1,
                            )
                        kt = kvo.tile([P, SB], BF16, name="kt")
                        nc.scalar.activation(
                            out=kt[:], in_=ps[:], func=ID,
                            bias=bk_sb[:, dt : dt + 1],
                        )
                        nc.sync.dma_start(
                            out=kT_loc[dt * P : (dt + 1) * P,
                                       tb * SB : (tb + 1) * SB],
                            in_=kt[:],
                        )
                nc.gpsimd.collective_compute(
                    "AllGather", mybir.AluOpType.bypass,
                    replica_groups=GROUPS,
                    ins=[kT_loc[:]], outs=[kT_all[:]],
                )

                # ---- phase V: V = y^T Wv + bv (own t-half) ------------
                for tt in range(NTT):
                    for db in range(2):
                        ps = qps.tile([P, SB], F32, name="v_ps", tag="qkvps")
                        for c in range(DC):
                            _mm(
                                nc, ps[:],
                                yT_sb[:, c, tt * P : (tt + 1) * P],
                                wv_res[:, db, c, :],
                                c == 0, c == DC - 1,
                            )
                        vt = kvo.tile([P, SB], BF16, name="vt")
                        nc.vector.tensor_add(
                            vt[:], ps[:], bv_bc[:, db * SB : (db + 1) * SB]
                        )
                        nc.sync.dma_start(
                            out=v_loc[tt * P : (tt + 1) * P,
                                      db * SB : (db + 1) * SB],
                            in_=vt[:],
                        )
                nc.gpsimd.collective_compute(
                    "AllGather", mybir.AluOpType.bypass,
                    replica_groups=GROUPS,
                    ins=[v_loc[:]], outs=[v_all[:]],
                )

                # ---- phase Q: Q^T = Wq^T x^T + bq (s-block outer) -----
                for sb in range(NSB):
                    for dt in range(DC):
                        ps = qps.tile([P, SB], F32, name="q_ps", tag="qkvps")
                        for c in range(DC):
                            _mm(
                                nc, ps[:],
                                wq_res[:, dt, c, :],
                                xT_sb[:, c, sb * SB : (sb + 1) * SB],
                                c == 0, c == DC - 1,
                            )
                        nc.scalar.activation(
                            out=QT_sb[:, dt, sb * SB : (sb + 1) * SB],
                            in_=ps[:], func=ID, bias=bq_sb[:, dt : dt + 1],
                        )

            # ---- resident K/V loads (after collectives) ---------------
            with (
                tc.tile_pool(name="att_res", bufs=1) as ares,
                tc.tile_pool(name="exp_pool", bufs=1) as expp,
                tc.tile_pool(name="tmp_pool", bufs=4) as tmpp,
                tc.tile_pool(name="sc_pool", bufs=2) as scp,
                tc.tile_pool(name="o_out", bufs=3) as oout,
                tc.tile_pool(name="aps", bufs=2, space="PSUM") as aps,
                tc.tile_pool(name="nps", bufs=4, space="PSUM") as npsp,
                tc.tile_pool(name="ops", bufs=2, space="PSUM") as ops,
            ):
                K_res = ares.tile([P, DC, S], BF16, name="K_res")
                V_res = ares.tile([P, NG, D], BF16, name="V_res")
                for r in range(2):
                    nc.sync.dma_start(
                        out=K_res[:, :, r * SC : (r + 1) * SC], in_=kT_all_r[r]
                    )
                for r in range(2):
                    nc.sync.dma_start(
                        out=V_res[:, r * NTT : (r + 1) * NTT, :], in_=v_all_r[r]
                    )

                # ---- attention + output, s-block outer ----------------
                for sb in range(NSB):
                    ssl = slice(sb * SB, (sb + 1) * SB)
                    ex = expp.tile([P, NG, SB], BF16, name="ex")
                    # pass1: logits -> exp, denominator
                    for g in range(NG):
                        att = aps.tile([P, SB], F32, name="att")
                        for c in range(DC):
                            _mm(
                                nc, att[:],
                                K_res[:, c, g * P : (g + 1) * P],
                                QT_sb[:, c, ssl],
                                c == 0, c == DC - 1,
                            )
                        tm = tmpp.tile([P, SB], F32, name="tm")
                        nc.vector.tensor_add(tm[:], att[:], adj_sb[:, g, ssl])
                        nc.scalar.activation(out=ex[:, g, :], in_=tm[:], func=EXP)
                        if g == 0:
                            nc.vector.tensor_copy(denacc[:, sb, :], ex[:, g, :])
                        else:
                            nc.vector.tensor_add(
                                denacc[:, sb, :], denacc[:, sb, :], ex[:, g, :]
                            )
                    nc.gpsimd.partition_all_reduce(
                        dsum[:], denacc[:, sb, :],
                        channels=P, reduce_op=bass_isa.ReduceOp.add,
                    )
                    nc.vector.reciprocal(recip_sb[0:1, sb, :], dsum[0:1, :])
                    nc.gpsimd.partition_broadcast(
                        rb[:, sb, :], recip_sb[0:1, sb, :], channels=P
                    )
                    # pass2: numerator accumulated in PSUM over all 16 t-tiles
                    sc_t = scp.tile([P, DC, SB], BF16, name="sc_t")
                    for dh in range(2):
                        nt = [
                            npsp.tile([P, SB], F32, name="np", tag="nt")
                            for _ in range(DC // 2)
                        ]
                        for g in range(NG):
                            for d4 in range(DC // 2):
                                _mm(
                                    nc, nt[d4][:],
                                    V_res[:, g,
                                          (dh * 4 + d4) * P : (dh * 4 + d4 + 1) * P],
                                    ex[:, g, :],
                                    g == 0, g == NG - 1,
                                )
                        for d4 in range(DC // 2):
                            nc.vector.tensor_copy(
                                sc_t[:, dh * 4 + d4, :], nt[d4][:]
                            )
                    # O-proj for this s-block; recip folded in post-matmul
                    for dt in range(DC):
                        po = ops.tile([P, SB], F32, name="po")
                        for c in range(DC):
                            _mm(
                                nc, po[:],
                                wo_res[:, dt, c, :],
                                sc_t[:, c, :],
                                c == 0, c == DC - 1,
                            )
                        tm2 = tmpp.tile([P, SB], F32, name="tm2", tag="tm2")
                        nc.vector.tensor_mul(tm2[:], po[:], rb[:, sb, :])
                        ot = oout.tile([P, SB], F32, name="ot")
                        nc.scalar.activation(
                            out=ot[:], in_=tm2[:], func=ID,
                            bias=bo_sb[:, dt : dt + 1],
                        )
                        nc.sync.dma_start(
                            out=outT[dt * P : (dt + 1) * P, ssl],
                            in_=ot[:],
                        )
    nc.compile()
    return nc


def _get_nc():
    if "nc" not in _CACHE:
        _CACHE["nc"] = build_nc()
    return _CACHE["nc"]


BF = ml_dtypes.bfloat16


def _tile_lhs(W):
    # [dt][p][c][col] = W[c*P+p, dt*P+col]
    return np.ascontiguousarray(
        W.reshape(DC, P, DC, P).transpose(2, 1, 0, 3)
    )


def kernel(x, y, adj, Wq, bq, Wk, bk, Wv, bv, Wo, bo, _trace=False):
    x = np.asarray(x, dtype=np.float32).astype(BF)
    y = np.asarray(y, dtype=np.float32).astype(BF)
    adj = np.asarray(adj, dtype=np.float32).astype(BF)
    Wq_h = _tile_lhs((np.asarray(Wq, np.float32) * NORM).astype(BF))
    Wk_h = _tile_lhs(np.asarray(Wk, np.float32).astype(BF))
    Wo_h = _tile_lhs(np.asarray(Wo, np.float32).astype(BF))
    # Wv as rhs tiles: [db][p][c][col] = Wv[c*P+p, db*SB+col]
    Wv_h = np.ascontiguousarray(
        np.asarray(Wv, np.float32).astype(BF)
        .reshape(DC, P, 2, SB).transpose(2, 1, 0, 3)
    )
    bq_s = np.asarray(bq, np.float32) * NORM
    bq_h = np.ascontiguousarray(bq_s.reshape(DC, P).T)
    bk_h = np.ascontiguousarray(np.asarray(bk, np.float32).reshape(DC, P).T)
    bo_h = np.ascontiguousarray(np.asarray(bo, np.float32).reshape(DC, P).T)
    bv_h = np.ascontiguousarray(np.asarray(bv, np.float32).reshape(1, D))

    in_maps = []
    for c in range(8):
        b, h = c // 2, c % 2
        ssl = slice(h * SC, (h + 1) * SC)
        in_maps.append(
            {
                "xT": np.ascontiguousarray(x[b, ssl, :].T),
                "yT": np.ascontiguousarray(y[b, ssl, :].T),
                "adjT": np.ascontiguousarray(adj[b, ssl, :].T),
                "Wq": Wq_h, "Wk": Wk_h, "Wv": Wv_h, "Wo": Wo_h,
                "bq": bq_h, "bk": bk_h, "bv": bv_h, "bo": bo_h,
            }
        )

    nc = _get_nc()
    res = run_bass_kernel_spmd(nc, in_maps, list(range(8)), trace=_trace)
    if _trace:
        _CACHE["last_exec_time_ns"] = res.exec_time_ns
        _CACHE["last_trace"] = (
            res.instructions_and_trace[1] if res.instructions_and_trace else None
        )

    out = np.empty((4, S, D), np.float32)
    for c in range(8):
        b, h = c // 2, c % 2
        out[b, h * SC : (h + 1) * SC, :] = res.results[c]["outT"].T
    return out


# revision 17
# speedup vs baseline: 1.0615x; 1.0615x over previous
"""Fused single-head cross-attention on 8 TRN2 NeuronCores (Bass/Tile).

Problem: out = (softmax(norm * (xWq+bq)(yWk+bk)^T + adj) @ (yWv+bv)) Wo + bo
Shapes: x,y [4, 2048, 1024], adj [4, 2048, 2048], all weights [1024, 1024].

Sharding: data-parallel over (batch, seq-half) -> 8 shards. Core c handles
batch b=c//2, query rows h*1024..(h+1)*1024 (h=c%2). K/V projections are
split across the core pair (each computes its own t-half) and exchanged
with two pair-wise 2MB AllGathers (K first, then V), hidden under the V/Q
projections. Collectives >2MB hit an RDH buffer cliff (4MB = 107us vs
2MB = 22us), so K and V are exchanged separately.

All matmul operands are bf16 (1 cyc/row on the PE, same as fp32r, but half
the DMA/SBUF/collective bytes); PSUM accumulation is fp32 throughout.
Inputs/weights are cast to bf16 on the host so every DMA moves half the
bytes. Biases stay fp32 (applied via ACT bias port / DVE add).

Attention phase is fully SBUF-resident (K, V, adj, Wo preloaded; zero DMA
during attention), s-block outer:
  pass1: for all 16 t-tiles: att = K^T Q (PSUM), +adj (DVE), exp (ACT,
         bf16 out), denominator accumulated on DVE.
  pass2: numerator accumulated IN PSUM across all 16 t-tiles (4 banks per
         d-half), evacuated once per s-block.
  O-proj per s-block; softmax 1/denom is folded in after the matmul
  (out = (Wo^T num) * recip + bo), so pass2 needs no pre-scaling.
softmax max-subtraction is skipped: logits are O(1) by construction.
"""
import sys

if "/opt/trn_rl_repo" not in sys.path:
    sys.path.insert(0, "/opt/trn_rl_repo")

import numpy as np
import ml_dtypes

import concourse.bass as bass
import concourse.bass_isa as bass_isa
import concourse.tile as tile
from concourse import bacc, mybir
from concourse.bass_utils import run_bass_kernel_spmd

P = 128
D = 1024
S = 2048
SC = 1024            # per-core query rows; also per-core K/V t-half
DC = D // P          # 8 feature chunks
SB = 512             # matmul moving free dim
NSB = SC // SB       # 2 s blocks
NG = S // P          # 16 global t-tiles
NTT = SC // P        # 8 local t-tiles (own half)
NORM = 1.0 / 32.0
GROUPS = [[0, 1], [2, 3], [4, 5], [6, 7]]

F32 = mybir.dt.float32
BF16 = mybir.dt.bfloat16
ID = mybir.ActivationFunctionType.Identity
EXP = mybir.ActivationFunctionType.Exp

_CACHE = {}


def _mm(nc, ps, lhsT, rhs, start, stop):
    nc.tensor.matmul(ps, lhsT=lhsT, rhs=rhs, start=start, stop=stop)


def build_nc():
    nc = bacc.Bacc("TRN2", target_bir_lowering=False, debug=False, num_devices=8)

    xT = nc.dram_tensor("xT", [D, SC], BF16, kind="ExternalInput")
    yT = nc.dram_tensor("yT", [D, SC], BF16, kind="ExternalInput")  # own t-half
    adjT = nc.dram_tensor("adjT", [S, SC], BF16, kind="ExternalInput")
    # weights pre-tiled on host: Wx_t[dt][p][c][col] = Wx[c*P+p, dt*P+col]
    Wq = nc.dram_tensor("Wq", [DC, P, DC, P], BF16, kind="ExternalInput")
    Wk = nc.dram_tensor("Wk", [DC, P, DC, P], BF16, kind="ExternalInput")
    Wo = nc.dram_tensor("Wo", [DC, P, DC, P], BF16, kind="ExternalInput")
    # Wv pre-tiled as rhs: Wv_t[db][p][c][col] = Wv[c*P+p, db*SB+col]
    Wv = nc.dram_tensor("Wv", [2, P, DC, SB], BF16, kind="ExternalInput")
    bq = nc.dram_tensor("bq", [P, DC], F32, kind="ExternalInput")
    bk = nc.dram_tensor("bk", [P, DC], F32, kind="ExternalInput")
    bv = nc.dram_tensor("bv", [1, D], F32, kind="ExternalInput")
    bo = nc.dram_tensor("bo", [P, DC], F32, kind="ExternalInput")
    outT = nc.dram_tensor("outT", [D, SC], F32, kind="ExternalOutput")

    # local K/V t-halves + pair-gathered full-S tensors (bf16, 2MB each)
    kT_loc = nc.dram_tensor("kT_loc", [D, SC], BF16)
    v_loc = nc.dram_tensor("v_loc", [SC, D], BF16)
    kT_all = nc.dram_tensor("kT_all", [2, D, SC], BF16)
    v_all = nc.dram_tensor("v_all", [2, SC, D], BF16)

    xT_r = xT.rearrange("(c p) s -> p c s", p=P)
    yT_r = yT.rearrange("(c p) t -> p c t", p=P)
    adj_r = adjT.rearrange("(g p) s -> p g s", p=P)
    kT_all_r = kT_all.rearrange("r (c p) t -> r p c t", p=P)
    v_all_r = v_all.rearrange("r (j p) d -> r p j d", p=P)
    Wk_r = Wk.rearrange("dt p c col -> p dt c col")
    Wq_r = Wq.rearrange("dt p c col -> p dt c col")
    Wo_r = Wo.rearrange("dt p c col -> p dt c col")
    Wv_r = Wv.rearrange("db p c col -> p db c col")

    with tile.TileContext(nc) as tc:
        with (
            nc.allow_low_precision(reason="bf16 attention, fp32 accumulation"),
            tc.tile_pool(name="res", bufs=1) as res,
        ):
            # ---- resident tiles --------------------------------------
            QT_sb = res.tile([P, DC, SC], BF16, name="QT_sb")
            adj_sb = res.tile([P, NG, SC], BF16, name="adj_sb")
            wo_res = res.tile([P, DC, DC, P], BF16, name="wo_res")
            denacc = res.tile([P, NSB, SB], F32, name="denacc")
            dsum = res.tile([P, SB], F32, name="dsum")
            recip_sb = res.tile([1, NSB, SB], F32, name="recip_sb")
            rb = res.tile([P, NSB, SB], F32, name="rb")
            bv_bc = res.tile([P, D], F32, name="bv_bc")
            bq_sb = res.tile([P, DC], F32, name="bq_sb")
            bk_sb = res.tile([P, DC], F32, name="bk_sb")
            bo_sb = res.tile([P, DC], F32, name="bo_sb")
            bv_sb = res.tile([1, D], F32, name="bv_sb")
            nc.sync.dma_start(out=bk_sb[:], in_=bk[:])
            nc.sync.dma_start(out=bv_sb[:], in_=bv[:])
            nc.sync.dma_start(out=bq_sb[:], in_=bq[:])
            nc.sync.dma_start(out=bo_sb[:], in_=bo[:])
            nc.gpsimd.partition_broadcast(bv_bc[:], bv_sb[0:1, :], channels=P)

            with (
                tc.tile_pool(name="qkv_in", bufs=1) as qkvp,
                tc.tile_pool(name="kv_out", bufs=3) as kvo,
                tc.tile_pool(name="qkv_ps", bufs=3, space="PSUM") as qps,
            ):
                yT_sb = qkvp.tile([P, DC, SC], BF16, name="yT_sb")
                xT_sb = qkvp.tile([P, DC, SC], BF16, name="xT_sb")
                wk_res = qkvp.tile([P, DC, DC, P], BF16, name="wk_res")
                wq_res = qkvp.tile([P, DC, DC, P], BF16, name="wq_res")
                wv_res = qkvp.tile([P, 2, DC, SB], BF16, name="wv_res")
                # DMA dispatch is FIFO *per ring* (~2us fixed + bytes/340GB/s
                # each, serial). Three rings exist: sync (SP HWDGE), scalar
                # (ACT HWDGE), gpsimd (SWDGE). Spread loads so no ring's
                # backlog ever gates compute:
                #   sync:   yT, wk, wv (K/V-proj critical path), K/V gathers,
                #           out stores
                #   scalar: xT, wq, adj, wo (needed later; dispatched up
                #           front, drains in the background)
                #   gpsimd: kt/vt stores feeding the collectives
                nc.sync.dma_start(out=yT_sb[:, :, 0:SB], in_=yT_r[:, :, 0:SB])
                nc.sync.dma_start(out=wk_res[:, 0], in_=Wk_r[:, 0])
                nc.sync.dma_start(out=yT_sb[:, :, SB:SC], in_=yT_r[:, :, SB:SC])
                nc.sync.dma_start(out=wk_res[:, 1:4], in_=Wk_r[:, 1:4])
                nc.sync.dma_start(out=wk_res[:, 4:8], in_=Wk_r[:, 4:8])
                nc.sync.dma_start(out=wv_res[:], in_=Wv_r[:])
                for i in range(2):
                    nc.sync.dma_start(
                        out=xT_sb[:, 4 * i : 4 * (i + 1), :],
                        in_=xT_r[:, 4 * i : 4 * (i + 1), :],
                    )
                for i in range(2):
                    nc.sync.dma_start(
                        out=wq_res[:, 4 * i : 4 * (i + 1)],
                        in_=Wq_r[:, 4 * i : 4 * (i + 1)],
                    )
                for i in range(2):
                    nc.sync.dma_start(
                        out=adj_sb[:, 8 * i : 8 * (i + 1), :],
                        in_=adj_r[:, 8 * i : 8 * (i + 1), :],
                    )
                for i in range(2):
                    nc.sync.dma_start(
                        out=wo_res[:, 4 * i : 4 * (i + 1)],
                        in_=Wo_r[:, 4 * i : 4 * (i + 1)],
                    )

                # ---- phase K: K^T = Wk^T y^T + bk (own t-half) -------
                for dt in range(DC):
                    for tb in range(NSB):
                        ps = qps.tile([P, SB], F32, name="k_ps", tag="qkvps")
                        for c in range(DC):
                            _mm(
                                nc, ps[:],
                                wk_res[:, dt, c, :],
                                yT_sb[:, c, tb * SB : (tb + 1) * SB],
                                c == 0, c == DC - 1,
                            )
                        kt = kvo.tile([P, SB], BF16, name="kt")
                        nc.scalar.activation(
                            out=kt[:], in_=ps[:], func=ID,
                            bias=bk_sb[:, dt : dt + 1],
                        )
                        nc.gpsimd.dma_start(
                            out=kT_loc[dt * P : (dt + 1) * P,
                                       tb * SB : (tb + 1) * SB],
                            in_=kt[:],
                        )
                nc.gpsimd.collective_compute(
                    "AllGather", mybir.AluOpType.bypass,
                    replica_groups=GROUPS,
                    ins=[kT_loc[:]], outs=[kT_all[:]],
                )

                # ---- phase V: V = y^T Wv + bv (own t-half) ------------
                for tt in range(NTT):
                    for db in range(2):
                        ps = qps.tile([P, SB], F32, name="v_ps", tag="qkvps")
                        for c in range(DC):
                            _mm(
                                nc, ps[:],
                                yT_sb[:, c, tt * P : (tt + 1) * P],
                                wv_res[:, db, c, :],
                                c == 0, c == DC - 1,
                            )
                        vt = kvo.tile([P, SB], BF16, name="vt")
                        nc.vector.tensor_add(
                            vt[:], ps[:], bv_bc[:, db * SB : (db + 1) * SB]
                        )
                        nc.gpsimd.dma_start(
                            out=v_loc[tt * P : (tt + 1) * P,
                                      db * SB : (db + 1) * SB],
                            in_=vt[:],
                        )
                nc.gpsimd.collective_compute(
                    "AllGather", mybir.AluOpType.bypass,
                    replica_groups=GROUPS,
                    ins=[v_loc[:]], outs=[v_all[:]],
                )

                # ---- phase Q: Q^T = Wq^T x^T + bq (s-block outer) -----
                for sb in range(NSB):
                    for dt in range(DC):
                        ps = qps.tile([P, SB], F32, name="q_ps", tag="qkvps")
                        for c in range(DC):
                            _mm(
                                nc, ps[:],
                                wq_res[:, dt, c, :],
                                xT_sb[:, c, sb * SB : (sb + 1) * SB],
                                c == 0, c == DC - 1,
                            )
                        nc.scalar.activation(
                            out=QT_sb[:, dt, sb * SB : (sb + 1) * SB],
                            in_=ps[:], func=ID, bias=bq_sb[:, dt : dt + 1],
                        )

            # ---- resident K/V loads (after collectives) ---------------
            with (
                tc.tile_pool(name="att_res", bufs=1) as ares,
                tc.tile_pool(name="exp_pool", bufs=1) as expp,
                tc.tile_pool(name="tmp_pool", bufs=4) as tmpp,
                tc.tile_pool(name="sc_pool", bufs=2) as scp,
                tc.tile_pool(name="o_out", bufs=3) as oout,
                tc.tile_pool(name="aps", bufs=2, space="PSUM") as aps,
                tc.tile_pool(name="nps", bufs=4, space="PSUM") as npsp,
                tc.tile_pool(name="ops", bufs=2, space="PSUM") as ops,
            ):
                K_res = ares.tile([P, DC, S], BF16, name="K_res")
                V_res = ares.tile([P, NG, D], BF16, name="V_res")
                for r in range(2):
                    nc.sync.dma_start(
                        out=K_res[:, :, r * SC : (r + 1) * SC], in_=kT_all_r[r]
                    )
                for r in range(2):
                    nc.sync.dma_start(
                        out=V_res[:, r * NTT : (r + 1) * NTT, :], in_=v_all_r[r]
                    )

                # ---- attention + output, s-block outer ----------------
                for sb in range(NSB):
                    ssl = slice(sb * SB, (sb + 1) * SB)
                    ex = expp.tile([P, NG, SB], BF16, name="ex")
                    # pass1: logits -> exp, denominator
                    for g in range(NG):
                        att = aps.tile([P, SB], F32, name="att")
                        for c in range(DC):
                            _mm(
                                nc, att[:],
                                K_res[:, c, g * P : (g + 1) * P],
                                QT_sb[:, c, ssl],
                                c == 0, c == DC - 1,
                            )
                        tm = tmpp.tile([P, SB], F32, name="tm")
                        nc.vector.tensor_add(tm[:], att[:], adj_sb[:, g, ssl])
                        nc.scalar.activation(out=ex[:, g, :], in_=tm[:], func=EXP)
                        if g == 0:
                            nc.vector.tensor_copy(denacc[:, sb, :], ex[:, g, :])
                        else:
                            nc.vector.tensor_add(
                                denacc[:, sb, :], denacc[:, sb, :], ex[:, g, :]
                            )
                    nc.gpsimd.partition_all_reduce(
                        dsum[:], denacc[:, sb, :],
                        channels=P, reduce_op=bass_isa.ReduceOp.add,
                    )
                    nc.vector.reciprocal(recip_sb[0:1, sb, :], dsum[0:1, :])
                    nc.gpsimd.partition_broadcast(
                        rb[:, sb, :], recip_sb[0:1, sb, :], channels=P
                    )
                    # pass2: numerator accumulated in PSUM over all 16 t-tiles
                    sc_t = scp.tile([P, DC, SB], BF16, name="sc_t")
                    for dh in range(2):
                        nt = [
                            npsp.tile([P, SB], F32, name="np", tag="nt")
                            for _ in range(DC // 2)
                        ]
                        for g in range(NG):
                            for d4 in range(DC // 2):
                                _mm(
                                    nc, nt[d4][:],
                                    V_res[:, g,
                                          (dh * 4 + d4) * P : (dh * 4 + d4 + 1) * P],
                                    ex[:, g, :],
                                    g == 0, g == NG - 1,
                                )
                        for d4 in range(DC // 2):
                            nc.vector.tensor_copy(
                                sc_t[:, dh * 4 + d4, :], nt[d4][:]
                            )
                    # O-proj for this s-block; recip folded in post-matmul
                    for dt in range(DC):
                        po = ops.tile([P, SB], F32, name="po")
                        for c in range(DC):
                            _mm(
                                nc, po[:],
                                wo_res[:, dt, c, :],
                                sc_t[:, c, :],
                                c == 0, c == DC - 1,
                            )
                        tm2 = tmpp.tile([P, SB], F32, name="tm2", tag="tm2")
                        nc.vector.tensor_mul(tm2[:], po[:], rb[:, sb, :])
                        ot = oout.tile([P, SB], F32, name="ot")
                        nc.scalar.activation(
                            out=ot[:], in_=tm2[:], func=ID,
                            bias=bo_sb[:, dt : dt + 1],
                        )
                        nc.sync.dma_start(
                            out=outT[dt * P : (dt + 1) * P, ssl],
                            in_=ot[:],
                        )
    nc.compile()
    return nc


def _get_nc():
    if "nc" not in _CACHE:
        _CACHE["nc"] = build_nc()
    return _CACHE["nc"]


BF = ml_dtypes.bfloat16


def _tile_lhs(W):
    # [dt][p][c][col] = W[c*P+p, dt*P+col]
    return np.ascontiguousarray(
        W.reshape(DC, P, DC, P).transpose(2, 1, 0, 3)
    )


def kernel(x, y, adj, Wq, bq, Wk, bk, Wv, bv, Wo, bo, _trace=False):
    x = np.asarray(x, dtype=np.float32).astype(BF)
    y = np.asarray(y, dtype=np.float32).astype(BF)
    adj = np.asarray(adj, dtype=np.float32).astype(BF)
    Wq_h = _tile_lhs((np.asarray(Wq, np.float32) * NORM).astype(BF))
    Wk_h = _tile_lhs(np.asarray(Wk, np.float32).astype(BF))
    Wo_h = _tile_lhs(np.asarray(Wo, np.float32).astype(BF))
    # Wv as rhs tiles: [db][p][c][col] = Wv[c*P+p, db*SB+col]
    Wv_h = np.ascontiguousarray(
        np.asarray(Wv, np.float32).astype(BF)
        .reshape(DC, P, 2, SB).transpose(2, 1, 0, 3)
    )
    bq_s = np.asarray(bq, np.float32) * NORM
    bq_h = np.ascontiguousarray(bq_s.reshape(DC, P).T)
    bk_h = np.ascontiguousarray(np.asarray(bk, np.float32).reshape(DC, P).T)
    bo_h = np.ascontiguousarray(np.asarray(bo, np.float32).reshape(DC, P).T)
    bv_h = np.ascontiguousarray(np.asarray(bv, np.float32).reshape(1, D))

    in_maps = []
    for c in range(8):
        b, h = c // 2, c % 2
        ssl = slice(h * SC, (h + 1) * SC)
        in_maps.append(
            {
                "xT": np.ascontiguousarray(x[b, ssl, :].T),
                "yT": np.ascontiguousarray(y[b, ssl, :].T),
                "adjT": np.ascontiguousarray(adj[b, ssl, :].T),
                "Wq": Wq_h, "Wk": Wk_h, "Wv": Wv_h, "Wo": Wo_h,
                "bq": bq_h, "bk": bk_h, "bv": bv_h, "bo": bo_h,
            }
        )

    nc = _get_nc()
    res = run_bass_kernel_spmd(nc, in_maps, list(range(8)), trace=_trace)
    if _trace:
        _CACHE["last_exec_time_ns"] = res.exec_time_ns
        _CACHE["last_trace"] = (
            res.instructions_and_trace[1] if res.instructions_and_trace else None
        )

    out = np.empty((4, S, D), np.float32)
    for c in range(8):
        b, h = c // 2, c % 2
        out[b, h * SC : (h + 1) * SC, :] = res.results[c]["outT"].T
    return out
